# revision 29
# baseline (speedup 1.0000x reference)
"""Trainium2 Bass kernel for nn_Encoder_6 (conv+GN+InterpLnr x3 -> biLSTM).

Self-contained: host-side prep (sharding, interp gather tables, weight
repacking) + Bass/Tile device kernel + output gather.

Data-parallel over 8 NeuronCores: 64 samples per core.

Device dataflow per core:
  - activations in [channel(partition), sample, time] layout, fp16
  - conv1d = 10-11 accumulating matmuls per sample-pair (taps x cin-chunks)
  - GN stats: ACT evac psum->SBUF, DVE square + tensor_reduce (batched)
  - normalize+ReLU = one ACT op per sample-half (per-partition scale/bias)
  - InterpLnr = gpsimd ap_gather (idx, idx+1) + DVE w1*g1+w2*g2 combine
    (weights pre-broadcast on host, idx tables host-computed)
  - biLSTM: gate-partition layout [4*32 gates, 8 chunks x 64 samples],
    each direction split into 8 time-chunks of 24 steps + 12-step warmup;
    tanh-only gates (sigmoid(x) = (1+tanh(x/2))/2 folded into DVE ops),
    cell state kept as W=2c, h kept as h2=2h, output scaled by 0.5.
"""
import sys
from contextlib import ExitStack

sys.path.insert(0, "/opt/trn_rl_repo")

import numpy as np

B = 512
N_CORES = 8
S = B // N_CORES          # samples per core
DIM_PIT = 257
C = 256                   # conv channels
T = 192                   # padded time
TH2 = 244                 # 24 zero | 2 halo | 192 data | 2 halo | 24 zero
XOFF = 26                 # column of t=0 in xbuf
GRP = 16                  # channels per group
DIM_NECK = 32
FREQ = 8
NT_OUT = 24               # output timesteps per direction
MIN_LEN_SEG = 19
MAX_NUM_SEG = 7
W64 = 64                  # 2*MAX_LEN_SEG
EPS = 1e-5
SG = 32                   # samples per stats group (2 groups per core)
NPAIR = 16                # sample pairs per stats group
NCHUNK = 8                # LSTM time chunks per direction
CH_LEN = T // NCHUNK      # 24
WARM = 24                 # LSTM warmup steps
NSTEP = CH_LEN + WARM     # 48

_cache = {}


# ---------------------------------------------------------------- host prep

def _interp_tables(scales_u, len_seg_raw, n):
    """Gather idx/w1/w2 per sample for one interp layer (numpy, exact)."""
    scales = scales_u.astype(np.float32) + np.float32(0.5)
    j = np.arange(W64, dtype=np.float32)
    idx_scaled = j[None, :] / scales[:, None]
    idx_fl = np.floor(idx_scaled)
    lam = idx_scaled - idx_fl
    len_seg = (len_seg_raw + MIN_LEN_SEG).astype(np.float32)[:, None]
    idx_mask = idx_fl < (len_seg - 1.0)
    ls = (len_seg_raw + MIN_LEN_SEG).reshape(n, MAX_NUM_SEG)
    offset = np.cumsum(ls, axis=-1)
    offset = np.pad(offset[:, :-1], ((0, 0), (1, 0))).reshape(-1, 1)
    idx_org = idx_fl + offset.astype(np.float32)
    mask = (idx_mask & (idx_org < (T - 1))).reshape(n, MAX_NUM_SEG * W64)
    idx_b = np.clip(idx_org.reshape(n, -1).astype(np.int32), 0, T - 2)
    lam_b = lam.reshape(n, -1)
    idx = np.zeros((n, T), np.int32)
    w1 = np.zeros((n, T), np.float32)
    w2 = np.zeros((n, T), np.float32)
    for b in range(n):
        js = np.nonzero(mask[b])[0][:T]
        k = len(js)
        idx[b, :k] = idx_b[b, js]
        w1[b, :k] = 1.0 - lam_b[b, js]
        w2[b, :k] = lam_b[b, js]
    return idx, w1, w2


def _wrap_idx(idx_lists):
    """[n, NI] int -> ap_gather wrapped layout [n, 128, NI//16] int16."""
    n, NI = idx_lists.shape
    wrapped = idx_lists.reshape(n, NI // 16, 16).transpose(0, 2, 1)
    out = np.tile(wrapped[:, None, :, :], (1, 8, 1, 1)).reshape(n, 128, NI // 16)
    return np.ascontiguousarray(out.astype(np.int16))


def _prep_host(inputs):
    """Build per-core input dicts. Returns list of 8 dicts."""
    x = np.asarray(inputs["x"], np.float32)
    scales = np.asarray(inputs["scales"], np.float32)
    lsr = np.asarray(inputs["len_seg_raw"], np.int32)

    # conv weights as lhsT tiles [l, chunk, tap, half, cin128, cout128]
    wconv = np.zeros((3, 2, 5, 2, 128, 128), np.float32)
    for l in range(3):
        w = np.asarray(inputs[f"conv{l}_w"], np.float32)  # [256, cin, 5]
        for cc in range(2):
            for k in range(5):
                for h in range(2):
                    wconv[l, cc, k, h] = w[h * 128:(h + 1) * 128,
                                           cc * 128:(cc + 1) * 128, k].T
    wconv = np.ascontiguousarray(wconv.astype(np.float16))
    # conv0 channel 256 as [5, 256] lhsT (k=tap)
    w0 = np.asarray(inputs["conv0_w"], np.float32)
    wc0e = np.ascontiguousarray(w0[:, 256, :].T.astype(np.float16))  # [5, 256]

    conv_bias = [np.asarray(inputs[f"conv{l}_b"], np.float32) for l in range(3)]
    assert all(np.abs(b).max() == 0.0 for b in conv_bias), \
        "nonzero conv bias not implemented in device kernel"

    gamma_t = np.stack([np.asarray(inputs[f"gn{l}_g"], np.float32).reshape(2, 128)
                        for l in range(3)])          # [3, 2, 128]
    beta_t = np.stack([np.asarray(inputs[f"gn{l}_b"], np.float32).reshape(2, 128)
                       for l in range(3)])
    # device folds A=gamma*rstd into the interp-output writeback and relu's
    # mean subtraction assumes relu(A(y-mean)) == A*relu(y-mean), i.e. A>0;
    # beta is folded nowhere, so it must be 0 (true for this model).
    assert np.abs(beta_t).max() == 0.0, "nonzero GN beta not supported"
    assert gamma_t.min() > 0.0, "non-positive GN gamma not supported"
    gamma_t = np.ascontiguousarray(gamma_t.transpose(2, 0, 1).reshape(128, 6))
    beta_t = np.ascontiguousarray(beta_t.transpose(2, 0, 1).reshape(128, 6))

    gind = np.zeros((128, 8), np.float32)
    for c in range(128):
        gind[c, c // 16] = 1.0
    gexp = np.ascontiguousarray(gind.T)               # [8, 128]

    # interp tables, all samples
    idx_all, w1_all, w2_all = [], [], []
    for l in range(3):
        idx, w1, w2 = _interp_tables(scales[l], lsr[l], B)
        idx_all.append(idx)
        w1_all.append(w1)
        w2_all.append(w2)

    # LSTM weights: gate order i,f,o,g in partition blocks of 32
    def reord(a):
        i_, f_, g_, o_ = np.split(a, 4, axis=0)
        return np.concatenate([i_, f_, o_, g_], axis=0)

    wihT = np.zeros((128, 2, 2, 128), np.float32)     # [cin128, cc, dir, gate]
    whhT = np.zeros((32, 2, 128), np.float32)         # [h, dir, gate]
    bias2 = np.zeros((128, 2), np.float32)            # [gate, dir] (pre-scaled)
    scale1 = np.zeros((128, 1), np.float32)
    scale1[0:96, 0] = 0.5
    scale1[96:128, 0] = 1.0
    for d, nm in enumerate(["f", "b"]):
        wi = reord(np.asarray(inputs[f"w_ih_{nm}"], np.float32))   # [128, 256]
        wh = reord(np.asarray(inputs[f"w_hh_{nm}"], np.float32))   # [128, 32]
        bb = reord((np.asarray(inputs[f"b_ih_{nm}"], np.float32)
                    + np.asarray(inputs[f"b_hh_{nm}"], np.float32))[:, None])[:, 0]
        for cc in range(2):
            wihT[:, cc, d, :] = wi[:, cc * 128:(cc + 1) * 128].T
        whhT[:, d, :] = wh.T * 0.5       # rhs is h2 = 2h
        bias2[:, d] = bb * scale1[:, 0]  # ACT computes tanh(scale*x + bias)
    wihT = np.ascontiguousarray(wihT.astype(np.float16))
    whhT = np.ascontiguousarray(whhT.astype(np.float16))

    in_maps = []
    for core in range(N_CORES):
        s0 = core * S
        xs = x[s0:s0 + S]                              # [S, 257, 192]
        xt = xs.transpose(1, 0, 2)                     # [257, S, 192]
        xa = np.zeros((128, S, TH2), np.float32)
        xb = np.zeros((128, S, TH2), np.float32)
        xa[:, :, XOFF:XOFF + T] = xt[:128]
        xb[:, :, XOFF:XOFF + T] = xt[128:256]
        xc = np.zeros((5, S, T), np.float32)
        x256 = xt[256]                                 # [S, 192]
        for k in range(5):
            sh = k - 2
            lo, hi = max(0, -sh), min(T, T - sh)
            xc[k, :, lo:hi] = x256[:, lo + sh:hi + sh]

        # banded interp matrices S[t_in, t_out] per (layer, sample), fp16
        wS = np.zeros((3, S, T, T), np.float16)
        bi = np.arange(S)[:, None]
        pj = np.arange(T)[None, :]
        for l in range(3):
            idx = idx_all[l][s0:s0 + S]
            Sm = np.zeros((S, T, T), np.float32)
            Sm[bi, idx, pj] = w1_all[l][s0:s0 + S]
            Sm[bi, idx + 1, pj] += w2_all[l][s0:s0 + S]
            wS[l] = Sm.astype(np.float16)

        in_maps.append({
            "xa": np.ascontiguousarray(xa.astype(np.float16)),
            "xb": np.ascontiguousarray(xb.astype(np.float16)),
            "xc": np.ascontiguousarray(xc.astype(np.float16)),
            "wconv": wconv,
            "wc0e": wc0e,
            "gamma_t": gamma_t,
            "beta_t": beta_t,
            "gind": gind,
            "gexp": gexp,
            "wS": np.ascontiguousarray(wS),
            "id128": np.eye(128, dtype=np.float16),
            "wihT": wihT,
            "whhT": whhT,
            "bias2": bias2,
            "scale1": scale1,
        })
    return in_maps


# ------------------------------------------------------------- device build

def _build(probe_layer=-1):
    import concourse.bass as bass
    import concourse.tile as tile
    from concourse import bacc, mybir

    f32 = mybir.dt.float32
    fp16 = mybir.dt.float16
    i16 = mybir.dt.int16
    AF = mybir.ActivationFunctionType
    OP = mybir.AluOpType
    AX = mybir.AxisListType

    nc = bacc.Bacc("TRN2", target_bir_lowering=False, debug=False,
                   enable_asserts=False, num_devices=N_CORES)

    # DRAM tensors
    d_xa = nc.dram_tensor("xa", [128, S, TH2], fp16, kind="ExternalInput")
    d_xb = nc.dram_tensor("xb", [128, S, TH2], fp16, kind="ExternalInput")
    d_xc = nc.dram_tensor("xc", [5, S, T], fp16, kind="ExternalInput")
    d_wconv = nc.dram_tensor("wconv", [3, 2, 5, 2, 128, 128], fp16,
                             kind="ExternalInput")
    d_wc0e = nc.dram_tensor("wc0e", [5, 256], fp16, kind="ExternalInput")
    d_gamma = nc.dram_tensor("gamma_t", [128, 6], f32, kind="ExternalInput")
    d_beta = nc.dram_tensor("beta_t", [128, 6], f32, kind="ExternalInput")
    d_gind = nc.dram_tensor("gind", [128, 8], f32, kind="ExternalInput")
    d_gexp = nc.dram_tensor("gexp", [8, 128], f32, kind="ExternalInput")
    d_wS = nc.dram_tensor("wS", [3, S, T, T], fp16, kind="ExternalInput")
    d_id128 = nc.dram_tensor("id128", [128, 128], fp16, kind="ExternalInput")
    d_wih = nc.dram_tensor("wihT", [128, 2, 2, 128], fp16, kind="ExternalInput")
    d_whh = nc.dram_tensor("whhT", [32, 2, 128], fp16, kind="ExternalInput")
    d_bias2 = nc.dram_tensor("bias2", [128, 2], f32, kind="ExternalInput")
    d_scale1 = nc.dram_tensor("scale1", [128, 1], f32, kind="ExternalInput")
    d_out = nc.dram_tensor("out", [S, NT_OUT, 64], f32, kind="ExternalOutput")
    d_probe = None
    if probe_layer >= 0:
        d_probe = nc.dram_tensor("probe", [2, 128, S, TH2], fp16,
                                 kind="ExternalOutput")

    es = ExitStack()
    with tile.TileContext(nc) as tc, es:
        consts = es.enter_context(tc.tile_pool(name="consts", bufs=1))
        xbufs = es.enter_context(tc.tile_pool(name="xbufs", bufs=1))

        # ---- constants
        t_xc = consts.tile([5, S, T], fp16)
        nc.sync.dma_start(out=t_xc[:], in_=d_xc[:, :, :])
        t_wc0e = consts.tile([5, 256], fp16)
        nc.sync.dma_start(out=t_wc0e[:], in_=d_wc0e[:, :])
        t_gamma = consts.tile([128, 6], f32)
        nc.sync.dma_start(out=t_gamma[:], in_=d_gamma[:, :])
        t_beta = consts.tile([128, 6], f32)
        nc.sync.dma_start(out=t_beta[:], in_=d_beta[:, :])
        t_gind = consts.tile([128, 8], f32)
        nc.sync.dma_start(out=t_gind[:], in_=d_gind[:, :])
        t_gexp = consts.tile([8, 128], f32)
        nc.sync.dma_start(out=t_gexp[:], in_=d_gexp[:, :])
        t_eps = consts.tile([8, 1], f32)
        nc.vector.memset(t_eps[:], EPS)
        t_ones1 = consts.tile([1, 128], fp16)
        nc.vector.memset(t_ones1[:], 1.0)
        t_id128 = consts.tile([128, 128], fp16)
        nc.sync.dma_start(out=t_id128[:], in_=d_id128[:, :])
        t_wih = consts.tile([128, 2, 2, 128], fp16)
        nc.sync.dma_start(out=t_wih[:], in_=d_wih[:, :, :, :])
        t_whh = consts.tile([32, 2, 128], fp16)
        nc.sync.dma_start(out=t_whh[:], in_=d_whh[:, :, :])
        t_bias2 = consts.tile([128, 2], f32)
        nc.sync.dma_start(out=t_bias2[:], in_=d_bias2[:, :])
        t_scale1 = consts.tile([128, 1], f32)
        nc.sync.dma_start(out=t_scale1[:], in_=d_scale1[:, :])

        # ---- input activations (xbuf rewritten in place by interp each layer)
        t_xa = xbufs.tile([128, S, TH2], fp16)
        t_xb = xbufs.tile([128, S, TH2], fp16)
        nc.sync.dma_start(out=t_xa[:], in_=d_xa[:, :, :])
        nc.sync.dma_start(out=t_xb[:], in_=d_xb[:, :, :])
        xbuf = [t_xa, t_xb]

        def mm(out, lhsT, rhs, start, stop, **kw):
            nc.tensor.matmul(out=out, lhsT=lhsT, rhs=rhs, start=start,
                             stop=stop, **kw)

        # ================= conv + GN + interp layers =================
        with ExitStack() as ces:
            wpool = ces.enter_context(tc.tile_pool(name="wpool", bufs=1))
            yb_p = ces.enter_context(tc.tile_pool(name="ybp", bufs=2))
            sq_p = ces.enter_context(tc.tile_pool(name="sqp", bufs=3))
            stats_p = ces.enter_context(tc.tile_pool(name="stats", bufs=2))
            small_p = ces.enter_context(tc.tile_pool(name="small", bufs=2))
            neg_p = ces.enter_context(tc.tile_pool(name="negp", bufs=2))
            sm_p = ces.enter_context(tc.tile_pool(name="smp", bufs=3))
            yt_p = ces.enter_context(tc.tile_pool(name="ytp", bufs=3))
            cpsum = ces.enter_context(
                tc.tile_pool(name="cpsum", bufs=3, space="PSUM"))
            stps = ces.enter_context(
                tc.tile_pool(name="stps", bufs=1, space="PSUM"))
            tps = ces.enter_context(
                tc.tile_pool(name="tps", bufs=2, space="PSUM"))
            sops = ces.enter_context(
                tc.tile_pool(name="sops", bufs=2, space="PSUM"))

            for l in range(3):
                t_wc = wpool.tile([128, 20, 128], fp16, tag="wconv")
                nc.sync.dma_start(
                    out=t_wc[:],
                    in_=bass.AP(tensor=d_wconv, offset=l * 20 * 128 * 128,
                                ap=[[128, 128], [128 * 128, 20], [1, 128]]))

                for grp in range(2):
                    s0g = grp * SG
                    sums = [stats_p.tile([128, SG], f32, tag=f"sums{h}",
                                         name=f"sums{h}") for h in range(2)]
                    qs = [stats_p.tile([128, SG], f32, tag=f"qs{h}",
                                       name=f"qs{h}") for h in range(2)]
                    ybt = [yb_p.tile([128, SG, T], fp16, tag=f"yb{h}",
                                     name=f"yb{h}") for h in range(2)]

                    # ---- phase 1: conv -> evac -> stats
                    for pp in range(NPAIR):
                        pr = s0g + 2 * pp
                        for h in range(2):
                            ps = cpsum.tile([128, 2, 256], f32, tag="cps")
                            ops = []
                            for cc in range(2):
                                for k in range(5):
                                    ops.append((
                                        t_wc[:, (cc * 5 + k) * 2 + h, :],
                                        xbuf[cc][:, pr:pr + 2,
                                                 XOFF - 2 + k:XOFF - 2 + k + T]))
                            if l == 0:
                                ops.append((
                                    t_wc0e[:, h * 128:(h + 1) * 128],
                                    t_xc[:, pr:pr + 2, :]))
                            for j, (lh, rh) in enumerate(ops):
                                mm(ps[:, :, 0:T], lh, rh, j == 0,
                                   j == len(ops) - 1)
                            nc.scalar.activation(
                                out=ybt[h][:, 2 * pp:2 * pp + 2, :],
                                in_=ps[:, :, 0:T], func=AF.Identity)
                            scr = sq_p.tile([128, 2, T], fp16, tag="sq")
                            nc.vector.tensor_tensor(
                                out=scr[:],
                                in0=ybt[h][:, 2 * pp:2 * pp + 2, :],
                                in1=ybt[h][:, 2 * pp:2 * pp + 2, :],
                                op=OP.mult)
                            nc.vector.tensor_reduce(
                                out=sums[h][:, 2 * pp:2 * pp + 2],
                                in_=ybt[h][:, 2 * pp:2 * pp + 2, :],
                                axis=AX.X, op=OP.add)
                            nc.vector.tensor_reduce(
                                out=qs[h][:, 2 * pp:2 * pp + 2],
                                in_=scr[:], axis=AX.X, op=OP.add)

                    # ---- phase 2: group stats -> A, -mean per half
                    AB = []
                    for h in range(2):
                        g1 = stps.tile([8, SG], f32, tag="gg")
                        mm(g1[:], t_gind[:], sums[h][:], True, True)
                        g2 = stps.tile([8, SG], f32, tag="gg")
                        mm(g2[:], t_gind[:], qs[h][:], True, True)
                        mean = small_p.tile([8, SG], f32, tag="mean")
                        nc.vector.tensor_scalar_mul(mean[:], g1[:],
                                                    1.0 / (GRP * T))
                        msq = small_p.tile([8, SG], f32, tag="msq")
                        nc.vector.tensor_tensor(out=msq[:], in0=mean[:],
                                                in1=mean[:], op=OP.mult)
                        var = small_p.tile([8, SG], f32, tag="var")
                        nc.vector.scalar_tensor_tensor(
                            out=var[:], in0=g2[:], scalar=1.0 / (GRP * T),
                            in1=msq[:], op0=OP.mult, op1=OP.subtract)
                        sd = small_p.tile([8, SG], f32, tag="sd")
                        nc.scalar.activation(out=sd[:], in_=var[:],
                                             func=AF.Sqrt,
                                             bias=t_eps[:, :1], scale=1.0)
                        rstd = small_p.tile([8, SG], f32, tag="rstd")
                        nc.vector.reciprocal(rstd[:], sd[:])
                        rp = stps.tile([128, SG], f32, tag="gg")
                        mm(rp[:], t_gexp[:], rstd[:], True, True)
                        mp = stps.tile([128, SG], f32, tag="gg")
                        mm(mp[:], t_gexp[:], mean[:], True, True)
                        At = small_p.tile([128, SG], f32, tag="A")
                        nc.vector.tensor_scalar_mul(
                            At[:], rp[:],
                            t_gamma[:, l * 2 + h:l * 2 + h + 1])
                        negmp = small_p.tile([128, SG], f32, tag="negmp")
                        nc.vector.tensor_scalar_mul(negmp[:], mp[:], -1.0)
                        AB.append((At, negmp))

                    # ---- phase 3: transpose(-mean), relu-evac, interp matmul
                    for pp in range(NPAIR):
                        pr = s0g + 2 * pp
                        s128 = sm_p.tile([128, 2, T], fp16, tag="s128")
                        nc.gpsimd.dma_start(
                            out=s128[:],
                            in_=d_wS[l, pr:pr + 2, 0:128, :].rearrange(
                                "s t w -> t s w"))
                        s64 = sm_p.tile([64, 2, T], fp16, tag="s64")
                        nc.gpsimd.dma_start(
                            out=s64[:],
                            in_=d_wS[l, pr:pr + 2, 128:192, :].rearrange(
                                "s t w -> t s w"))
                        yts = []
                        for i in range(2):
                            sl = 2 * pp + i
                            pt = tps.tile([128, 2, 256], fp16, tag="pt",
                                          name="pt")
                            for h in range(2):
                                At, negmp = AB[h]
                                ymc = sq_p.tile([128, T], fp16, tag="ymc")
                                nc.vector.tensor_scalar_add(
                                    ymc[:], ybt[h][:, sl, :],
                                    negmp[:, sl:sl + 1])
                                nc.tensor.transpose(
                                    out=pt[:, 0, h * 128:(h + 1) * 128],
                                    in_=ymc[:, 0:128],
                                    identity=t_id128[:])
                                nc.tensor.transpose(
                                    out=pt[0:64, 1, h * 128:(h + 1) * 128],
                                    in_=ymc[:, 128:192],
                                    identity=t_id128[:])
                            yt = yt_p.tile([128, 2, 256], fp16, tag="yt")
                            nc.scalar.activation(out=yt[:], in_=pt[:],
                                                 func=AF.Relu)
                            yts.append(yt)
                        for ch in range(2):
                            At = AB[ch][0]
                            sop = sops.tile([128, 2, T], f32, tag="so",
                                            name="sop")
                            for i in range(2):
                                mm(sop[:, i, :],
                                   yts[i][:, 0, ch * 128:(ch + 1) * 128],
                                   s128[:, i, :], True, False)
                                mm(sop[:, i, :],
                                   yts[i][0:64, 1, ch * 128:(ch + 1) * 128],
                                   s64[:, i, :], False, True)
                            for i in range(2):
                                sl = 2 * pp + i
                                nc.vector.tensor_scalar_mul(
                                    xbuf[ch][:, pr + i, XOFF:XOFF + T],
                                    sop[:, i, :], At[:, sl:sl + 1])

                if probe_layer == l:
                    for h in range(2):
                        nc.sync.dma_start(out=d_probe[h, :, :, :],
                                          in_=xbuf[h][:, :, :])

        # ======================= biLSTM =======================
        # Exact full-length recurrence, 192 steps, two direction chains.
        # gate-partition layout: psum [128 gates, 8 steps, 64 samples].
        # tanh-only gates: sigmoid(z) = (1+tanh(z/2))/2 folded into DVE ops;
        # cell state stored as W=2c, hidden as h2=2h; outputs scaled 0.5.
        LBLK = 8
        lsb = es.enter_context(tc.tile_pool(name="lstm_sbuf", bufs=1))
        lst = es.enter_context(tc.tile_pool(name="lstm_tmp", bufs=3))
        t_OUT = [lsb.tile([S, NT_OUT, 32], f32, name=f"outd{d}")
                 for d in range(2)]
        # W state at partition base 32 (f-gate base), tc at base 64 (o-gate
        # base) -- walrus requires equal base partitions for SBUF+SBUF
        # binary DVE ops.
        W0 = lsb.tile([64, 2, S], fp16, name="w0")
        H0 = [lsb.tile([32, S], fp16, name=f"h0_{d}") for d in range(2)]
        nc.vector.memset(W0[32:64, :, :], 0.0)
        for d in range(2):
            nc.vector.memset(H0[d][:], 0.0)

        with tc.tile_pool(name="lpsum", bufs=2, space="PSUM") as lpsum, \
             tc.tile_pool(name="ltp", bufs=2, space="PSUM") as ltp:

            Wprev = [W0, W0]
            Hprev = [H0[0], H0[1]]

            def xw_block(k, d):
                """gate preacts for steps 8k..8k+7 of dir d -> one bank.

                Slots hold ascending time: fwd slot j = t 8k+j, bwd block k
                covers t in [184-8k, 191-8k], slot j = t 184-8k+j."""
                ps = lpsum.tile([128, LBLK, S], f32, tag=f"xw{d}",
                                name=f"xw{d}")
                t0 = 8 * k if d == 0 else 184 - 8 * k
                for cc in range(2):
                    apx = xbuf[cc][:, :, :]
                    rhs = bass.AP(
                        tensor=apx.tensor,
                        offset=apx.offset + XOFF + t0,
                        ap=[list(apx.ap[0]), [1, LBLK], [TH2, S]])
                    mm(ps[:], t_wih[:, cc, d, :], rhs, cc == 0, False,
                       skip_group_check=True)
                return ps

            psq = [[xw_block(0, d), None] for d in range(2)]

            for n in range(T):
                k, j = n // LBLK, n % LBLK
                slots = [j, LBLK - 1 - j]   # fwd ascending, bwd descending
                for d in range(2):
                    ps = psq[d][k % 2]
                    mm(ps[:, slots[d], :], t_whh[:, d, :], Hprev[d][:],
                       False, True, skip_group_check=True)
                if j == 1 and k + 1 < T // LBLK:
                    for d in range(2):
                        psq[d][(k + 1) % 2] = xw_block(k + 1, d)
                G = []
                for d in range(2):
                    ps = psq[d][k % 2]
                    tg = lst.tile([128, S], fp16, tag=f"G{d}", name=f"G{d}")
                    nc.scalar.activation(out=tg[:], in_=ps[:, slots[d], :],
                                         func=AF.Tanh,
                                         bias=t_bias2[:, d:d + 1],
                                         scale=t_scale1[:, 0:1])
                    G.append(tg)
                Wn2 = lst.tile([64, 2, S], fp16, tag="W2", name="W2")
                for d in range(2):
                    tg = G[d]
                    gc = lst.tile([32, S], fp16, tag=f"gc{d}")
                    nc.scalar.activation(out=gc[:], in_=tg[96:128, :],
                                         func=AF.Identity)
                    u = lst.tile([32, S], fp16, tag=f"u{d}")
                    nc.vector.scalar_tensor_tensor(
                        out=u[:], in0=tg[32:64, :], scalar=1.0,
                        in1=Wprev[d][32:64, d, :], op0=OP.add, op1=OP.mult)
                    v = lst.tile([32, S], fp16, tag=f"v{d}")
                    nc.vector.scalar_tensor_tensor(
                        out=v[:], in0=tg[0:32, :], scalar=1.0,
                        in1=gc[:], op0=OP.add, op1=OP.mult)
                    nc.vector.scalar_tensor_tensor(
                        out=Wn2[32:64, d, :], in0=u[:], scalar=0.5,
                        in1=v[:], op0=OP.mult, op1=OP.add)
                # one tanh(c) for both directions
                tc_ = lst.tile([96, 2, S], fp16, tag="tc2")
                nc.scalar.activation(out=tc_[64:96, :, :],
                                     in_=Wn2[32:64, :, :],
                                     func=AF.Tanh, scale=0.5)
                for d in range(2):
                    tg = G[d]
                    Hn = lst.tile([32, S], fp16, tag=f"H{d}", name=f"H{d}")
                    nc.vector.scalar_tensor_tensor(
                        out=Hn[:], in0=tg[64:96, :], scalar=1.0,
                        in1=tc_[64:96, d, :], op0=OP.add, op1=OP.mult)
                    Wprev[d] = Wn2
                    Hprev[d] = Hn

                if n % 8 == 7:
                    for d in range(2):
                        ot = n // 8 if d == 0 else 23 - n // 8
                        pt = ltp.tile([64, 32], fp16, tag="tp", name="tp")
                        nc.tensor.transpose(
                            out=pt[:], in_=Hprev[d][:],
                            identity=t_id128[0:32, 0:32])
                        nc.vector.tensor_scalar_mul(
                            t_OUT[d][:, ot, :], pt[:], 0.5)

        nc.sync.dma_start(out=d_out[:, :, 0:32], in_=t_OUT[0][:])
        nc.sync.dma_start(out=d_out[:, :, 32:64], in_=t_OUT[1][:])

    nc.compile()
    return nc


def _get_nc(probe_layer=-1):
    key = ("nc", probe_layer)
    if key not in _cache:
        _cache[key] = _build(probe_layer)
    return _cache[key]


def run_on_cores(inputs, probe_layer=-1, trace=False):
    """Build (cached), run on 8 cores; returns BassKernelResults."""
    from concourse.bass_utils import run_bass_kernel_spmd

    nc = _get_nc(probe_layer)
    in_maps = _prep_host(inputs)
    last_exc = None
    for _ in range(3):
        try:
            res = run_bass_kernel_spmd(nc, in_maps,
                                       core_ids=list(range(N_CORES)),
                                       trace=trace)
            return res
        except Exception as e:  # transient NRT errors happen; retry
            last_exc = e
    raise last_exc


def assemble_output(res):
    out = np.zeros((B, NT_OUT, 64), np.float32)
    for core in range(N_CORES):
        s0 = core * S
        out[s0:s0 + S] = res.results[core]["out"]
    return out


def kernel(**inputs):
    res = run_on_cores(inputs)
    return assemble_output(res)


# revision 32
# speedup vs baseline: 1.0557x; 1.0557x over previous
"""Trainium2 Bass kernel for nn_Encoder_6 (conv+GN+InterpLnr x3 -> biLSTM).

Self-contained: host-side prep (sharding, interp gather tables, weight
repacking) + Bass/Tile device kernel + output gather.

Data-parallel over 8 NeuronCores: 64 samples per core.

Device dataflow per core:
  - activations in [channel(partition), sample, time] layout, fp16
  - conv1d = 10-11 accumulating matmuls per sample-pair (taps x cin-chunks)
  - GN stats: ACT evac psum->SBUF, DVE square + tensor_reduce (batched)
  - normalize+ReLU = one ACT op per sample-half (per-partition scale/bias)
  - InterpLnr = gpsimd ap_gather (idx, idx+1) + DVE w1*g1+w2*g2 combine
    (weights pre-broadcast on host, idx tables host-computed)
  - biLSTM: gate-partition layout [4*32 gates, 8 chunks x 64 samples],
    each direction split into 8 time-chunks of 24 steps + 12-step warmup;
    tanh-only gates (sigmoid(x) = (1+tanh(x/2))/2 folded into DVE ops),
    cell state kept as W=2c, h kept as h2=2h, output scaled by 0.5.
"""
import sys
from contextlib import ExitStack

sys.path.insert(0, "/opt/trn_rl_repo")

import numpy as np

B = 512
N_CORES = 8
S = B // N_CORES          # samples per core
DIM_PIT = 257
C = 256                   # conv channels
T = 192                   # padded time
TH2 = 244                 # 24 zero | 2 halo | 192 data | 2 halo | 24 zero
XOFF = 26                 # column of t=0 in xbuf
GRP = 16                  # channels per group
DIM_NECK = 32
FREQ = 8
NT_OUT = 24               # output timesteps per direction
MIN_LEN_SEG = 19
MAX_NUM_SEG = 7
W64 = 64                  # 2*MAX_LEN_SEG
EPS = 1e-5
SG = 32                   # samples per stats group (2 groups per core)
NPAIR = 16                # sample pairs per stats group
NCHUNK = 8                # LSTM time chunks per direction
CH_LEN = T // NCHUNK      # 24
WARM = 24                 # LSTM warmup steps
NSTEP = CH_LEN + WARM     # 48

_cache = {}


# ---------------------------------------------------------------- host prep

def _interp_tables(scales_u, len_seg_raw, n):
    """Gather idx/w1/w2 per sample for one interp layer (numpy, exact)."""
    scales = scales_u.astype(np.float32) + np.float32(0.5)
    j = np.arange(W64, dtype=np.float32)
    idx_scaled = j[None, :] / scales[:, None]
    idx_fl = np.floor(idx_scaled)
    lam = idx_scaled - idx_fl
    len_seg = (len_seg_raw + MIN_LEN_SEG).astype(np.float32)[:, None]
    idx_mask = idx_fl < (len_seg - 1.0)
    ls = (len_seg_raw + MIN_LEN_SEG).reshape(n, MAX_NUM_SEG)
    offset = np.cumsum(ls, axis=-1)
    offset = np.pad(offset[:, :-1], ((0, 0), (1, 0))).reshape(-1, 1)
    idx_org = idx_fl + offset.astype(np.float32)
    mask = (idx_mask & (idx_org < (T - 1))).reshape(n, MAX_NUM_SEG * W64)
    idx_b = np.clip(idx_org.reshape(n, -1).astype(np.int32), 0, T - 2)
    lam_b = lam.reshape(n, -1)
    idx = np.zeros((n, T), np.int32)
    w1 = np.zeros((n, T), np.float32)
    w2 = np.zeros((n, T), np.float32)
    for b in range(n):
        js = np.nonzero(mask[b])[0][:T]
        k = len(js)
        idx[b, :k] = idx_b[b, js]
        w1[b, :k] = 1.0 - lam_b[b, js]
        w2[b, :k] = lam_b[b, js]
    return idx, w1, w2


def _wrap_idx(idx_lists):
    """[n, NI] int -> ap_gather wrapped layout [n, 128, NI//16] int16."""
    n, NI = idx_lists.shape
    wrapped = idx_lists.reshape(n, NI // 16, 16).transpose(0, 2, 1)
    out = np.tile(wrapped[:, None, :, :], (1, 8, 1, 1)).reshape(n, 128, NI // 16)
    return np.ascontiguousarray(out.astype(np.int16))


def _prep_host(inputs):
    """Build per-core input dicts. Returns list of 8 dicts."""
    x = np.asarray(inputs["x"], np.float32)
    scales = np.asarray(inputs["scales"], np.float32)
    lsr = np.asarray(inputs["len_seg_raw"], np.int32)

    # conv weights as lhsT tiles [l, chunk, tap, half, cin128, cout128]
    wconv = np.zeros((3, 2, 5, 2, 128, 128), np.float32)
    for l in range(3):
        w = np.asarray(inputs[f"conv{l}_w"], np.float32)  # [256, cin, 5]
        for cc in range(2):
            for k in range(5):
                for h in range(2):
                    wconv[l, cc, k, h] = w[h * 128:(h + 1) * 128,
                                           cc * 128:(cc + 1) * 128, k].T
    wconv = np.ascontiguousarray(wconv.astype(np.float16))
    # conv0 channel 256 as [5, 256] lhsT (k=tap)
    w0 = np.asarray(inputs["conv0_w"], np.float32)
    wc0e = np.ascontiguousarray(w0[:, 256, :].T.astype(np.float16))  # [5, 256]

    conv_bias = [np.asarray(inputs[f"conv{l}_b"], np.float32) for l in range(3)]
    assert all(np.abs(b).max() == 0.0 for b in conv_bias), \
        "nonzero conv bias not implemented in device kernel"

    gamma_t = np.stack([np.asarray(inputs[f"gn{l}_g"], np.float32).reshape(2, 128)
                        for l in range(3)])          # [3, 2, 128]
    beta_t = np.stack([np.asarray(inputs[f"gn{l}_b"], np.float32).reshape(2, 128)
                       for l in range(3)])
    # device folds A=gamma*rstd into the interp-output writeback and relu's
    # mean subtraction assumes relu(A(y-mean)) == A*relu(y-mean), i.e. A>0;
    # beta is folded nowhere, so it must be 0 (true for this model).
    assert np.abs(beta_t).max() == 0.0, "nonzero GN beta not supported"
    assert gamma_t.min() > 0.0, "non-positive GN gamma not supported"
    gamma_t = np.ascontiguousarray(gamma_t.transpose(2, 0, 1).reshape(128, 6))
    beta_t = np.ascontiguousarray(beta_t.transpose(2, 0, 1).reshape(128, 6))

    gind = np.zeros((128, 8), np.float32)
    for c in range(128):
        gind[c, c // 16] = 1.0
    gexp = np.ascontiguousarray(gind.T)               # [8, 128]

    # interp tables, all samples
    idx_all, w1_all, w2_all = [], [], []
    for l in range(3):
        idx, w1, w2 = _interp_tables(scales[l], lsr[l], B)
        idx_all.append(idx)
        w1_all.append(w1)
        w2_all.append(w2)

    # LSTM weights: gate order i,f,o,g in partition blocks of 32
    def reord(a):
        i_, f_, g_, o_ = np.split(a, 4, axis=0)
        return np.concatenate([i_, f_, o_, g_], axis=0)

    wihT = np.zeros((128, 2, 2, 128), np.float32)     # [cin128, cc, dir, gate]
    whhT = np.zeros((32, 2, 128), np.float32)         # [h, dir, gate]
    bias2 = np.zeros((128, 2), np.float32)            # [gate, dir] (pre-scaled)
    scale1 = np.zeros((128, 1), np.float32)
    scale1[0:96, 0] = 0.5
    scale1[96:128, 0] = 1.0
    for d, nm in enumerate(["f", "b"]):
        wi = reord(np.asarray(inputs[f"w_ih_{nm}"], np.float32))   # [128, 256]
        wh = reord(np.asarray(inputs[f"w_hh_{nm}"], np.float32))   # [128, 32]
        bb = reord((np.asarray(inputs[f"b_ih_{nm}"], np.float32)
                    + np.asarray(inputs[f"b_hh_{nm}"], np.float32))[:, None])[:, 0]
        for cc in range(2):
            wihT[:, cc, d, :] = wi[:, cc * 128:(cc + 1) * 128].T
        whhT[:, d, :] = wh.T * 0.5       # rhs is h2 = 2h
        bias2[:, d] = bb * scale1[:, 0]  # ACT computes tanh(scale*x + bias)
    wihT = np.ascontiguousarray(wihT.astype(np.float16))
    whhT = np.ascontiguousarray(whhT.astype(np.float16))

    in_maps = []
    for core in range(N_CORES):
        s0 = core * S
        xs = x[s0:s0 + S]                              # [S, 257, 192]
        xt = xs.transpose(1, 0, 2)                     # [257, S, 192]
        xa = np.zeros((128, S, TH2), np.float32)
        xb = np.zeros((128, S, TH2), np.float32)
        xa[:, :, XOFF:XOFF + T] = xt[:128]
        xb[:, :, XOFF:XOFF + T] = xt[128:256]
        xc = np.zeros((5, S, T), np.float32)
        x256 = xt[256]                                 # [S, 192]
        for k in range(5):
            sh = k - 2
            lo, hi = max(0, -sh), min(T, T - sh)
            xc[k, :, lo:hi] = x256[:, lo + sh:hi + sh]

        # banded interp matrices S[t_in, t_out] per (layer, sample), fp16
        wS = np.zeros((3, S, T, T), np.float16)
        bi = np.arange(S)[:, None]
        pj = np.arange(T)[None, :]
        for l in range(3):
            idx = idx_all[l][s0:s0 + S]
            Sm = np.zeros((S, T, T), np.float32)
            Sm[bi, idx, pj] = w1_all[l][s0:s0 + S]
            Sm[bi, idx + 1, pj] += w2_all[l][s0:s0 + S]
            wS[l] = Sm.astype(np.float16)

        in_maps.append({
            "xa": np.ascontiguousarray(xa.astype(np.float16)),
            "xb": np.ascontiguousarray(xb.astype(np.float16)),
            "xc": np.ascontiguousarray(xc.astype(np.float16)),
            "wconv": wconv,
            "wc0e": wc0e,
            "gamma_t": gamma_t,
            "beta_t": beta_t,
            "gind": gind,
            "gexp": gexp,
            "wS": np.ascontiguousarray(wS),
            "id128": np.eye(128, dtype=np.float16),
            "wihT": wihT,
            "whhT": whhT,
            "bias2": bias2,
            "scale1": scale1,
        })
    return in_maps


# ------------------------------------------------------------- device build

def _build(probe_layer=-1):
    import concourse.bass as bass
    import concourse.tile as tile
    from concourse import bacc, mybir

    f32 = mybir.dt.float32
    fp16 = mybir.dt.float16
    i16 = mybir.dt.int16
    AF = mybir.ActivationFunctionType
    OP = mybir.AluOpType
    AX = mybir.AxisListType

    nc = bacc.Bacc("TRN2", target_bir_lowering=False, debug=False,
                   enable_asserts=False, num_devices=N_CORES)

    # DRAM tensors
    d_xa = nc.dram_tensor("xa", [128, S, TH2], fp16, kind="ExternalInput")
    d_xb = nc.dram_tensor("xb", [128, S, TH2], fp16, kind="ExternalInput")
    d_xc = nc.dram_tensor("xc", [5, S, T], fp16, kind="ExternalInput")
    d_wconv = nc.dram_tensor("wconv", [3, 2, 5, 2, 128, 128], fp16,
                             kind="ExternalInput")
    d_wc0e = nc.dram_tensor("wc0e", [5, 256], fp16, kind="ExternalInput")
    d_gamma = nc.dram_tensor("gamma_t", [128, 6], f32, kind="ExternalInput")
    d_beta = nc.dram_tensor("beta_t", [128, 6], f32, kind="ExternalInput")
    d_gind = nc.dram_tensor("gind", [128, 8], f32, kind="ExternalInput")
    d_gexp = nc.dram_tensor("gexp", [8, 128], f32, kind="ExternalInput")
    d_wS = nc.dram_tensor("wS", [3, S, T, T], fp16, kind="ExternalInput")
    d_id128 = nc.dram_tensor("id128", [128, 128], fp16, kind="ExternalInput")
    d_wih = nc.dram_tensor("wihT", [128, 2, 2, 128], fp16, kind="ExternalInput")
    d_whh = nc.dram_tensor("whhT", [32, 2, 128], fp16, kind="ExternalInput")
    d_bias2 = nc.dram_tensor("bias2", [128, 2], f32, kind="ExternalInput")
    d_scale1 = nc.dram_tensor("scale1", [128, 1], f32, kind="ExternalInput")
    d_out = nc.dram_tensor("out", [S, NT_OUT, 64], f32, kind="ExternalOutput")
    d_probe = None
    if probe_layer >= 0:
        d_probe = nc.dram_tensor("probe", [2, 128, S, TH2], fp16,
                                 kind="ExternalOutput")

    es = ExitStack()
    with tile.TileContext(nc) as tc, es:
        consts = es.enter_context(tc.tile_pool(name="consts", bufs=1))
        xbufs = es.enter_context(tc.tile_pool(name="xbufs", bufs=1))

        # ---- constants
        t_xc = consts.tile([5, S, T], fp16)
        nc.sync.dma_start(out=t_xc[:], in_=d_xc[:, :, :])
        t_wc0e = consts.tile([5, 256], fp16)
        nc.sync.dma_start(out=t_wc0e[:], in_=d_wc0e[:, :])
        t_gamma = consts.tile([128, 6], f32)
        nc.sync.dma_start(out=t_gamma[:], in_=d_gamma[:, :])
        t_beta = consts.tile([128, 6], f32)
        nc.sync.dma_start(out=t_beta[:], in_=d_beta[:, :])
        t_gind = consts.tile([128, 8], f32)
        nc.sync.dma_start(out=t_gind[:], in_=d_gind[:, :])
        t_gexp = consts.tile([8, 128], f32)
        nc.sync.dma_start(out=t_gexp[:], in_=d_gexp[:, :])
        t_eps = consts.tile([8, 1], f32)
        nc.vector.memset(t_eps[:], EPS)
        t_ones1 = consts.tile([1, 128], fp16)
        nc.vector.memset(t_ones1[:], 1.0)
        t_id128 = consts.tile([128, 128], fp16)
        nc.sync.dma_start(out=t_id128[:], in_=d_id128[:, :])
        t_wih = consts.tile([128, 2, 2, 128], fp16)
        nc.sync.dma_start(out=t_wih[:], in_=d_wih[:, :, :, :])
        t_whh = consts.tile([32, 2, 128], fp16)
        nc.sync.dma_start(out=t_whh[:], in_=d_whh[:, :, :])
        t_bias2 = consts.tile([128, 2], f32)
        nc.sync.dma_start(out=t_bias2[:], in_=d_bias2[:, :])
        t_scale1 = consts.tile([128, 1], f32)
        nc.sync.dma_start(out=t_scale1[:], in_=d_scale1[:, :])

        # ---- input activations (xbuf rewritten in place by interp each layer)
        t_xa = xbufs.tile([128, S, TH2], fp16)
        t_xb = xbufs.tile([128, S, TH2], fp16)
        nc.sync.dma_start(out=t_xa[:], in_=d_xa[:, :, :])
        nc.sync.dma_start(out=t_xb[:], in_=d_xb[:, :, :])
        xbuf = [t_xa, t_xb]

        def mm(out, lhsT, rhs, start, stop, **kw):
            nc.tensor.matmul(out=out, lhsT=lhsT, rhs=rhs, start=start,
                             stop=stop, **kw)

        # ================= conv + GN + interp layers =================
        with ExitStack() as ces:
            wpool = ces.enter_context(tc.tile_pool(name="wpool", bufs=1))
            yb_p = ces.enter_context(tc.tile_pool(name="ybp", bufs=2))
            sq_p = ces.enter_context(tc.tile_pool(name="sqp", bufs=3))
            stats_p = ces.enter_context(tc.tile_pool(name="stats", bufs=2))
            small_p = ces.enter_context(tc.tile_pool(name="small", bufs=2))
            neg_p = ces.enter_context(tc.tile_pool(name="negp", bufs=2))
            sm_p = ces.enter_context(tc.tile_pool(name="smp", bufs=3))
            yt_p = ces.enter_context(tc.tile_pool(name="ytp", bufs=3))
            cpsum = ces.enter_context(
                tc.tile_pool(name="cpsum", bufs=3, space="PSUM"))
            stps = ces.enter_context(
                tc.tile_pool(name="stps", bufs=1, space="PSUM"))
            tps = ces.enter_context(
                tc.tile_pool(name="tps", bufs=2, space="PSUM"))
            sops = ces.enter_context(
                tc.tile_pool(name="sops", bufs=2, space="PSUM"))

            for l in range(3):
                t_wc = wpool.tile([128, 20, 128], fp16, tag="wconv")
                nc.sync.dma_start(
                    out=t_wc[:],
                    in_=bass.AP(tensor=d_wconv, offset=l * 20 * 128 * 128,
                                ap=[[128, 128], [128 * 128, 20], [1, 128]]))

                for grp in range(2):
                    s0g = grp * SG
                    sums = [stats_p.tile([128, SG], f32, tag=f"sums{h}",
                                         name=f"sums{h}") for h in range(2)]
                    qs = [stats_p.tile([128, SG], f32, tag=f"qs{h}",
                                       name=f"qs{h}") for h in range(2)]
                    ybt = [yb_p.tile([128, SG, T], fp16, tag=f"yb{h}",
                                     name=f"yb{h}") for h in range(2)]

                    # ---- phase 1: conv -> evac -> stats
                    for pp in range(NPAIR):
                        pr = s0g + 2 * pp
                        for h in range(2):
                            ps = cpsum.tile([128, 2, 256], f32, tag="cps")
                            ops = []
                            for cc in range(2):
                                for k in range(5):
                                    ops.append((
                                        t_wc[:, (cc * 5 + k) * 2 + h, :],
                                        xbuf[cc][:, pr:pr + 2,
                                                 XOFF - 2 + k:XOFF - 2 + k + T]))
                            if l == 0:
                                ops.append((
                                    t_wc0e[:, h * 128:(h + 1) * 128],
                                    t_xc[:, pr:pr + 2, :]))
                            for j, (lh, rh) in enumerate(ops):
                                mm(ps[:, :, 0:T], lh, rh, j == 0,
                                   j == len(ops) - 1)
                            nc.scalar.activation(
                                out=ybt[h][:, 2 * pp:2 * pp + 2, :],
                                in_=ps[:, :, 0:T], func=AF.Identity)
                            scr = sq_p.tile([128, 2, T], fp16, tag="sq")
                            nc.vector.tensor_tensor(
                                out=scr[:],
                                in0=ybt[h][:, 2 * pp:2 * pp + 2, :],
                                in1=ybt[h][:, 2 * pp:2 * pp + 2, :],
                                op=OP.mult)
                            nc.vector.tensor_reduce(
                                out=sums[h][:, 2 * pp:2 * pp + 2],
                                in_=ybt[h][:, 2 * pp:2 * pp + 2, :],
                                axis=AX.X, op=OP.add)
                            nc.vector.tensor_reduce(
                                out=qs[h][:, 2 * pp:2 * pp + 2],
                                in_=scr[:], axis=AX.X, op=OP.add)

                    # ---- phase 2: group stats -> A, -mean per half
                    AB = []
                    for h in range(2):
                        g1 = stps.tile([8, SG], f32, tag="gg")
                        mm(g1[:], t_gind[:], sums[h][:], True, True)
                        g2 = stps.tile([8, SG], f32, tag="gg")
                        mm(g2[:], t_gind[:], qs[h][:], True, True)
                        mean = small_p.tile([8, SG], f32, tag="mean")
                        nc.vector.tensor_scalar_mul(mean[:], g1[:],
                                                    1.0 / (GRP * T))
                        msq = small_p.tile([8, SG], f32, tag="msq")
                        nc.vector.tensor_tensor(out=msq[:], in0=mean[:],
                                                in1=mean[:], op=OP.mult)
                        var = small_p.tile([8, SG], f32, tag="var")
                        nc.vector.scalar_tensor_tensor(
                            out=var[:], in0=g2[:], scalar=1.0 / (GRP * T),
                            in1=msq[:], op0=OP.mult, op1=OP.subtract)
                        sd = small_p.tile([8, SG], f32, tag="sd")
                        nc.scalar.activation(out=sd[:], in_=var[:],
                                             func=AF.Sqrt,
                                             bias=t_eps[:, :1], scale=1.0)
                        rstd = small_p.tile([8, SG], f32, tag="rstd")
                        nc.vector.reciprocal(rstd[:], sd[:])
                        rp = stps.tile([128, SG], f32, tag="gg")
                        mm(rp[:], t_gexp[:], rstd[:], True, True)
                        mp = stps.tile([128, SG], f32, tag="gg")
                        mm(mp[:], t_gexp[:], mean[:], True, True)
                        At = small_p.tile([128, SG], f32, tag="A")
                        nc.vector.tensor_scalar_mul(
                            At[:], rp[:],
                            t_gamma[:, l * 2 + h:l * 2 + h + 1])
                        negmp = small_p.tile([128, SG], f32, tag="negmp")
                        nc.vector.tensor_scalar_mul(negmp[:], mp[:], -1.0)
                        AB.append((At, negmp))

                    # ---- phase 3: transpose(-mean), relu-evac, interp matmul
                    for pp in range(NPAIR):
                        pr = s0g + 2 * pp
                        s128 = sm_p.tile([128, 2, T], fp16, tag="s128")
                        nc.gpsimd.dma_start(
                            out=s128[:],
                            in_=d_wS[l, pr:pr + 2, 0:128, :].rearrange(
                                "s t w -> t s w"))
                        s64 = sm_p.tile([64, 2, T], fp16, tag="s64")
                        nc.gpsimd.dma_start(
                            out=s64[:],
                            in_=d_wS[l, pr:pr + 2, 128:192, :].rearrange(
                                "s t w -> t s w"))
                        yts = []
                        for i in range(2):
                            sl = 2 * pp + i
                            pt = tps.tile([128, 2, 256], fp16, tag="pt",
                                          name="pt")
                            for h in range(2):
                                At, negmp = AB[h]
                                ymc = sq_p.tile([128, T], fp16, tag="ymc")
                                # balance mean-subtract across ACT and DVE
                                if h == 0:
                                    nc.scalar.activation(
                                        out=ymc[:], in_=ybt[h][:, sl, :],
                                        func=AF.Identity,
                                        bias=negmp[:, sl:sl + 1], scale=1.0)
                                else:
                                    nc.vector.tensor_scalar_add(
                                        ymc[:], ybt[h][:, sl, :],
                                        negmp[:, sl:sl + 1])
                                nc.tensor.transpose(
                                    out=pt[:, 0, h * 128:(h + 1) * 128],
                                    in_=ymc[:, 0:128],
                                    identity=t_id128[:])
                                nc.tensor.transpose(
                                    out=pt[0:64, 1, h * 128:(h + 1) * 128],
                                    in_=ymc[:, 128:192],
                                    identity=t_id128[:])
                            yt = yt_p.tile([128, 2, 256], fp16, tag="yt")
                            nc.scalar.activation(out=yt[:], in_=pt[:],
                                                 func=AF.Relu)
                            yts.append(yt)
                        for ch in range(2):
                            At = AB[ch][0]
                            sop = sops.tile([128, 2, T], f32, tag="so",
                                            name="sop")
                            for i in range(2):
                                mm(sop[:, i, :],
                                   yts[i][:, 0, ch * 128:(ch + 1) * 128],
                                   s128[:, i, :], True, False)
                                mm(sop[:, i, :],
                                   yts[i][0:64, 1, ch * 128:(ch + 1) * 128],
                                   s64[:, i, :], False, True)
                            for i in range(2):
                                sl = 2 * pp + i
                                nc.vector.tensor_scalar_mul(
                                    xbuf[ch][:, pr + i, XOFF:XOFF + T],
                                    sop[:, i, :], At[:, sl:sl + 1])

                if probe_layer == l:
                    for h in range(2):
                        nc.sync.dma_start(out=d_probe[h, :, :, :],
                                          in_=xbuf[h][:, :, :])

        # ======================= biLSTM =======================
        # Exact full-length recurrence, 192 steps, two direction chains.
        # gate-partition layout: psum [128 gates, 8 steps, 64 samples].
        # tanh-only gates: sigmoid(z) = (1+tanh(z/2))/2 folded into DVE ops;
        # cell state stored as W=2c, hidden as h2=2h; outputs scaled 0.5.
        LBLK = 8
        lsb = es.enter_context(tc.tile_pool(name="lstm_sbuf", bufs=1))
        lst = es.enter_context(tc.tile_pool(name="lstm_tmp", bufs=3))
        t_OUT = [lsb.tile([S, NT_OUT, 32], f32, name=f"outd{d}")
                 for d in range(2)]
        # W state at partition base 32 (f-gate base), tc at base 64 (o-gate
        # base) -- walrus requires equal base partitions for SBUF+SBUF
        # binary DVE ops.
        W0 = [lsb.tile([64, S], fp16, name=f"w0_{d}") for d in range(2)]
        H0 = [lsb.tile([32, S], fp16, name=f"h0_{d}") for d in range(2)]
        for d in range(2):
            nc.vector.memset(W0[d][32:64, :], 0.0)
            nc.vector.memset(H0[d][:], 0.0)

        with tc.tile_pool(name="lpsum", bufs=2, space="PSUM") as lpsum, \
             tc.tile_pool(name="ltp", bufs=2, space="PSUM") as ltp:

            Wprev = [W0[0], W0[1]]
            Hprev = [H0[0], H0[1]]

            def xw_block(k, d):
                """gate preacts for steps 8k..8k+7 of dir d -> one bank.

                Slots hold ascending time: fwd slot j = t 8k+j, bwd block k
                covers t in [184-8k, 191-8k], slot j = t 184-8k+j."""
                ps = lpsum.tile([128, LBLK, S], f32, tag=f"xw{d}",
                                name=f"xw{d}")
                t0 = 8 * k if d == 0 else 184 - 8 * k
                for cc in range(2):
                    apx = xbuf[cc][:, :, :]
                    rhs = bass.AP(
                        tensor=apx.tensor,
                        offset=apx.offset + XOFF + t0,
                        ap=[list(apx.ap[0]), [1, LBLK], [TH2, S]])
                    mm(ps[:], t_wih[:, cc, d, :], rhs, cc == 0, False,
                       skip_group_check=True)
                return ps

            psq = [[xw_block(0, d), None] for d in range(2)]

            for n in range(T):
                k, j = n // LBLK, n % LBLK
                slots = [j, LBLK - 1 - j]   # fwd ascending, bwd descending
                for d in range(2):
                    ps = psq[d][k % 2]
                    mm(ps[:, slots[d], :], t_whh[:, d, :], Hprev[d][:],
                       False, True, skip_group_check=True)
                if j == 1 and k + 1 < T // LBLK:
                    for d in range(2):
                        psq[d][(k + 1) % 2] = xw_block(k + 1, d)
                G = []
                for d in range(2):
                    ps = psq[d][k % 2]
                    tg = lst.tile([128, S], fp16, tag=f"G{d}", name=f"G{d}")
                    nc.scalar.activation(out=tg[:], in_=ps[:, slots[d], :],
                                         func=AF.Tanh,
                                         bias=t_bias2[:, d:d + 1],
                                         scale=t_scale1[:, 0:1])
                    G.append(tg)
                for d in range(2):
                    tg = G[d]
                    gc = lst.tile([32, S], fp16, tag=f"gc{d}")
                    nc.scalar.activation(out=gc[:], in_=tg[96:128, :],
                                         func=AF.Identity)
                    u = lst.tile([32, S], fp16, tag=f"u{d}")
                    nc.vector.scalar_tensor_tensor(
                        out=u[:], in0=tg[32:64, :], scalar=1.0,
                        in1=Wprev[d][32:64, :], op0=OP.add, op1=OP.mult)
                    v = lst.tile([32, S], fp16, tag=f"v{d}")
                    nc.vector.scalar_tensor_tensor(
                        out=v[:], in0=tg[0:32, :], scalar=1.0,
                        in1=gc[:], op0=OP.add, op1=OP.mult)
                    Wn = lst.tile([64, S], fp16, tag=f"W{d}", name=f"W{d}")
                    nc.vector.scalar_tensor_tensor(
                        out=Wn[32:64, :], in0=u[:], scalar=0.5,
                        in1=v[:], op0=OP.mult, op1=OP.add)
                    tc_ = lst.tile([96, S], fp16, tag=f"tc{d}")
                    nc.scalar.activation(out=tc_[64:96, :], in_=Wn[32:64, :],
                                         func=AF.Tanh, scale=0.5)
                    Hn = lst.tile([32, S], fp16, tag=f"H{d}", name=f"H{d}")
                    nc.vector.scalar_tensor_tensor(
                        out=Hn[:], in0=tg[64:96, :], scalar=1.0,
                        in1=tc_[64:96, :], op0=OP.add, op1=OP.mult)
                    Wprev[d] = Wn
                    Hprev[d] = Hn

                if n % 8 == 7:
                    for d in range(2):
                        ot = n // 8 if d == 0 else 23 - n // 8
                        pt = ltp.tile([64, 32], fp16, tag="tp", name="tp")
                        nc.tensor.transpose(
                            out=pt[:], in_=Hprev[d][:],
                            identity=t_id128[0:32, 0:32])
                        nc.vector.tensor_scalar_mul(
                            t_OUT[d][:, ot, :], pt[:], 0.5)

        nc.sync.dma_start(out=d_out[:, :, 0:32], in_=t_OUT[0][:])
        nc.sync.dma_start(out=d_out[:, :, 32:64], in_=t_OUT[1][:])

    nc.compile()
    return nc


def _get_nc(probe_layer=-1):
    key = ("nc", probe_layer)
    if key not in _cache:
        _cache[key] = _build(probe_layer)
    return _cache[key]


def run_on_cores(inputs, probe_layer=-1, trace=False):
    """Build (cached), run on 8 cores; returns BassKernelResults."""
    from concourse.bass_utils import run_bass_kernel_spmd

    nc = _get_nc(probe_layer)
    in_maps = _prep_host(inputs)
    last_exc = None
    for _ in range(3):
        try:
            res = run_bass_kernel_spmd(nc, in_maps,
                                       core_ids=list(range(N_CORES)),
                                       trace=trace)
            return res
        except Exception as e:  # transient NRT errors happen; retry
            last_exc = e
    raise last_exc


def assemble_output(res):
    out = np.zeros((B, NT_OUT, 64), np.float32)
    for core in range(N_CORES):
        s0 = core * S
        out[s0:s0 + S] = res.results[core]["out"]
    return out


def kernel(**inputs):
    res = run_on_cores(inputs)
    return assemble_output(res)
